# revision 1
# baseline (speedup 1.0000x reference)
"""Causal multi-head attention (B=2,T=2048,C=1024,H=16,Ca=64) on 8 trn2 cores.

Sharding: the 32 (batch, head) pairs are split across 8 cores — core c gets
batch b = c//4 and heads [4g, 4g+4) where g = c%4.  Each core computes its
heads' attention plus the partial output projection through its 256-row slice
of w_o; the host sums the 4 partials per batch.

Per-core layouts (everything keeps the contraction dim on partitions):
  xT   [8,128,2048]  x[b].T c-chunked
  wq/wk[2,8,128,128] per head-pair, per c-chunk, cols = [h0 64 | h1 64]
  wv   [8,128,256]   4 heads concatenated
  wo   [2,128,1024]  rows 256g..256g+256 of w_o, c_local-chunked
  out  [16,128,1024] partial output, t-blocked

On-chip: Q^T,K^T [128(2 heads),2048]; V natural [s,a] with a ones column
appended so the A@V matmul also emits the softmax row-sums l[t]; scores are
computed transposed (S^T[s,t]) so softmax needs no cross-partition reduction
and no max-subtraction (logits are bounded: |s*scale| < ~4).
"""

import math
import sys

import numpy as np

for _p in ("/opt/trn_rl_repo",):
    if _p not in sys.path:
        sys.path.insert(0, _p)

import concourse.bass as bass
from concourse import bacc
import concourse.mybir as mybir
from concourse.bass import ts
from concourse.tile import TileContext
from concourse.bass_utils import run_bass_kernel_spmd
from contextlib import ExitStack

F32 = mybir.dt.float32
F32R = mybir.dt.float32r
AF = mybir.ActivationFunctionType

B, T, C = 2, 2048, 1024
H, CA = 16, 64
SCALE = 1.0 / math.sqrt(CA)
NCORES = 8
HPC = 4          # heads per core
TB = T // 128    # 16 t-blocks of 128
TC = T // 512    # 4 t-chunks of 512
CK = C // 128    # 8 c-chunks




def build_nc():
    nc = bacc.Bacc()
    xT = nc.declare_dram_parameter("xT", [CK, 128, T], F32R, isOutput=False)
    wq = nc.declare_dram_parameter("wq", [2, CK, 128, 128], F32R, isOutput=False)
    wk = nc.declare_dram_parameter("wk", [2, CK, 128, 128], F32R, isOutput=False)
    wv = nc.declare_dram_parameter("wv", [CK, 128, 2 * 128], F32R, isOutput=False)
    wo = nc.declare_dram_parameter("wo", [2, 128, C], F32R, isOutput=False)
    mask_d = nc.declare_dram_parameter("mask", [128, 4, 512], F32R, isOutput=False)
    ones_d = nc.declare_dram_parameter("ones", [128, 64], F32R, isOutput=False)
    out = nc.declare_dram_parameter("out", [TB, 128, C], F32, isOutput=True)

    with TileContext(nc) as tc, ExitStack() as ctx:
        const = ctx.enter_context(tc.tile_pool(name="const", bufs=1))
        persist = ctx.enter_context(tc.tile_pool(name="persist", bufs=1))

        # 0/1 causal masks for the 4 diagonal-band shifts (S^T layout [s,t]):
        # keep (1.0) where 128*d + p <= f, else 0.  Host-computed.
        mask = const.tile([128, 4, 512], F32R)
        nc.scalar.dma_start(mask[:], mask_d[:])
        ones_sb = const.tile([128, 64], F32R)
        nc.gpsimd.dma_start(ones_sb[:], ones_d[:])
        ones1 = ones_sb[0:1, :]

        q_sb = [persist.tile([128, T], F32R, tag=f"q{p}", name=f"q{p}") for p in range(2)]
        k_sb = [persist.tile([128, T], F32R, tag=f"k{p}", name=f"k{p}") for p in range(2)]
        # V natural [s,a] per head, t-blocked, with ones column at a=64
        v_sb = persist.tile([128, HPC, TB, 65], F32R, tag="v")
        nc.sync.dma_start(
            v_sb[:, :, :, 64],
            ones_d[:].rearrange("p (h b) -> p h b", h=HPC),
        )
        y_sb = [persist.tile([128, T], F32R, tag=f"y{p}", name=f"y{p}") for p in range(2)]
        wo_sb = persist.tile([128, 2, C], F32R, tag="wo")
        for cl in range(2):
            nc.gpsimd.dma_start(wo_sb[:, cl, :], wo[cl])

        # ---------------- Phase B/C: projections ----------------
        with ExitStack() as pbc:
            xw = pbc.enter_context(tc.tile_pool(name="xw", bufs=1))
            ps_qk = pbc.enter_context(tc.tile_pool(name="ps_qk", bufs=4, space="PSUM"))
            ps_v = pbc.enter_context(tc.tile_pool(name="ps_v", bufs=3, space="PSUM"))

            xT_sb = xw.tile([128, CK, T], F32R, tag="xT")
            wq_sb = xw.tile([128, 2, CK, 128], F32R, tag="wq")
            wk_sb = xw.tile([128, 2, CK, 128], F32R, tag="wk")
            wv_sb = xw.tile([128, CK, 256], F32R, tag="wv")
            # weights for pair 0 first (first matmuls need them), x chunks
            # round-robined over issuing engines so queues run in parallel
            engs = [nc.sync, nc.scalar, nc.gpsimd]
            nc.sync.dma_start(xT_sb[:, 0, :], xT[0])
            nc.scalar.dma_start(wq_sb[:, 0, 0, :], wq[0, 0])
            for ck in range(1, CK):
                engs[ck % 3].dma_start(wq_sb[:, 0, ck, :], wq[0, ck])
            for ck in range(1, CK):
                engs[ck % 3].dma_start(xT_sb[:, ck, :], xT[ck])
            for ck in range(CK):
                engs[(ck + 1) % 3].dma_start(wk_sb[:, 0, ck, :], wk[0, ck])
                engs[(ck + 2) % 3].dma_start(wq_sb[:, 1, ck, :], wq[1, ck])
                engs[ck % 3].dma_start(wk_sb[:, 1, ck, :], wk[1, ck])
                engs[(ck + 1) % 3].dma_start(wv_sb[:, ck, :], wv[ck])

            # Q^T / K^T: [128(2 heads), T]
            for p in range(2):
                for w_s, dst in ((wq_sb, q_sb), (wk_sb, k_sb)):
                    pst = [ps_qk.tile([128, 512], F32, tag="qk", name="qkps") for _ in range(TC)]
                    for ck in range(CK):
                        for tcn in range(TC):
                            nc.tensor.matmul(
                                pst[tcn][:],
                                lhsT=(w_s[:, p, ck, :]),
                                rhs=(xT_sb[:, ck, ts(tcn, 512)]),
                                start=(ck == 0), stop=(ck == CK - 1),
                            )
                    for tcn in range(TC):
                        nc.vector.tensor_copy(dst[p][:, ts(tcn, 512)], pst[tcn][:])

            # V natural: [s(=t) blocks, 4*64]
            for tb in range(TB):
                vps = ps_v.tile([128, 256], F32, tag="v")
                for ck in range(CK):
                    nc.tensor.matmul(
                        vps[:],
                        lhsT=(xT_sb[:, ck, ts(tb, 128)]),
                        rhs=(wv_sb[:, ck, :]),
                        start=(ck == 0), stop=(ck == CK - 1),
                    )
                nc.vector.tensor_copy(
                    v_sb[:, :, tb, 0:64],
                    vps[:].rearrange("p (h a) -> p h a", h=HPC),
                )

        # ---------------- Phase D + E: attention and output projection ----
        # tcn-outer so the projection for finished t-chunks overlaps attention
        with ExitStack() as pd:
            pp = pd.enter_context(tc.tile_pool(name="pp", bufs=10))
            sm = pd.enter_context(tc.tile_pool(name="sm", bufs=4))
            ob = pd.enter_context(tc.tile_pool(name="ob", bufs=3))
            ps_s = pd.enter_context(tc.tile_pool(name="ps_s", bufs=2, space="PSUM"))
            ps_y = pd.enter_context(tc.tile_pool(name="ps_y", bufs=2, space="PSUM"))
            ps_o = pd.enter_context(tc.tile_pool(name="ps_o", bufs=2, space="PSUM"))

            def proj_block(tb):
                ot = ob.tile([128, C], F32, tag="o", name="ot")
                for cc in range(2):
                    ops_ = ps_o.tile([128, 512], F32, tag="o", name="ops")
                    for cl in range(2):
                        nc.tensor.matmul(
                            ops_[:],
                            lhsT=(y_sb[cl][:, ts(tb, 128)]),
                            rhs=(wo_sb[:, cl, ts(cc, 512)]),
                            start=(cl == 0), stop=(cl == 1),
                        )
                    nc.vector.tensor_copy(ot[:, ts(cc, 512)], ops_[:])
                nc.sync.dma_start(out[tb], ot[:])

            for tcn in range(TC):
                nsb = 4 * tcn + 4
                for p in range(2):
                    for hl in range(2):
                        h = 2 * p + hl
                        b0 = 64 * hl
                        yps = ps_y.tile([128, 512], F32, tag="y", name="yps")
                        for sb2 in range(0, nsb, 2):
                            sps = ps_s.tile([128, 1024], F32, tag="s", name="sps")
                            for j in range(2):
                                nc.tensor.matmul(
                                    sps[:, ts(j, 512)],
                                    lhsT=(k_sb[p][b0:b0 + 64, ts(sb2 + j, 128)]),
                                    rhs=(q_sb[p][b0:b0 + 64, ts(tcn, 512)]),
                                    start=True, stop=True,
                                )
                            pb = pp.tile([128, 1024], F32R, tag="pb", name="pb")
                            nc.scalar.activation(pb[:], sps[:], AF.Exp, scale=SCALE)
                            for j in range(2):
                                d = sb2 + j - 4 * tcn
                                if d >= 0:
                                    w = 128 * (d + 1)
                                    o = 512 * j
                                    nc.vector.tensor_mul(
                                        pb[:, o:o + w], pb[:, o:o + w],
                                        mask[:, d, :w])
                            for j in range(2):
                                nc.tensor.matmul(
                                    yps[0:65, :],
                                    lhsT=(v_sb[:, h, sb2 + j, :]),
                                    rhs=(pb[:, ts(j, 512)]),
                                    start=(sb2 + j == 0), stop=(sb2 + j == nsb - 1),
                                )
                        # normalize: y /= l (l = row 64 of yps)
                        lrow = sm.tile([1, 512], F32R, tag="l", name="lrow")
                        nc.vector.tensor_copy(lrow[:], yps[64:65, :])
                        bps = ps_o.tile([128, 512], F32, tag="o", name="bps")
                        nc.tensor.matmul(
                            bps[0:64, :], lhsT=(ones1[:]), rhs=(lrow[:]),
                            start=True, stop=True,
                        )
                        rb = sm.tile([64, 512], F32, tag="r", name="rb")
                        nc.vector.reciprocal(rb[:], bps[0:64, :])
                        nc.vector.tensor_mul(
                            y_sb[p][b0:b0 + 64, ts(tcn, 512)],
                            yps[0:64, :], rb[:],
                        )
                # project the 4 t-blocks of this finished chunk
                for tb in range(4 * tcn, 4 * tcn + 4):
                    proj_block(tb)

    nc.compile()
    return nc


_NC = None


def _get_nc():
    global _NC
    if _NC is None:
        _NC = build_nc()
    return _NC


def _mask_arr():
    p = np.arange(128)[:, None, None]
    d = np.arange(4)[None, :, None]
    f = np.arange(512)[None, None, :]
    return np.ascontiguousarray((128 * d + p <= f).astype(np.float32))


def make_in_maps(x, w_q, w_k, w_v, w_o):
    x = np.asarray(x, dtype=np.float32)
    w_q = np.asarray(w_q, dtype=np.float32)
    w_k = np.asarray(w_k, dtype=np.float32)
    w_v = np.asarray(w_v, dtype=np.float32)
    w_o = np.asarray(w_o, dtype=np.float32)
    in_maps = []
    for c in range(NCORES):
        b, g = c // 4, c % 4
        hs = [4 * g + i for i in range(HPC)]
        xT = np.ascontiguousarray(x[b].T).reshape(CK, 128, T)
        wq_a = np.stack([
            np.concatenate([w_q[hs[2 * p]], w_q[hs[2 * p + 1]]], axis=1).reshape(CK, 128, 128)
            for p in range(2)
        ])
        wk_a = np.stack([
            np.concatenate([w_k[hs[2 * p]], w_k[hs[2 * p + 1]]], axis=1).reshape(CK, 128, 128)
            for p in range(2)
        ])
        wv_a = np.concatenate([w_v[h] for h in hs], axis=1).reshape(CK, 128, 256)
        wo_a = w_o[256 * g:256 * (g + 1)].reshape(2, 128, C)
        in_maps.append(dict(
            mask=_mask_arr(),
            ones=np.ones((128, 64), np.float32),
            xT=np.ascontiguousarray(xT),
            wq=np.ascontiguousarray(wq_a),
            wk=np.ascontiguousarray(wk_a),
            wv=np.ascontiguousarray(wv_a),
            wo=np.ascontiguousarray(wo_a),
        ))
    return in_maps


def gather_out(results):
    acc = [np.zeros((T, C), np.float64) for _ in range(B)]
    for c in range(NCORES):
        acc[c // 4] += results[c]["out"].reshape(T, C).astype(np.float64)
    return np.stack([a.astype(np.float32) for a in acc])


def run(x, w_q, w_k, w_v, w_o, trace=False, **spmd_kwargs):
    nc = _get_nc()
    in_maps = make_in_maps(x, w_q, w_k, w_v, w_o)
    res = run_bass_kernel_spmd(nc, in_maps, list(range(NCORES)), trace=trace,
                               **spmd_kwargs)
    return gather_out(res.results), res


def kernel(x, w_q, w_k, w_v, w_o):
    out, _ = run(x, w_q, w_k, w_v, w_o)
    return out



# revision 16
# speedup vs baseline: 1.2670x; 1.2670x over previous
"""Causal multi-head attention (B=2,T=2048,C=1024,H=16,Ca=64) on 8 trn2 cores.

Sharding: the 32 (batch, head) pairs are split across 8 cores - core c gets
batch b = c//4 and heads [4g, 4g+4) where g = c%4.  Each core computes its
heads' attention plus the partial output projection through its 256-row slice
of w_o; the host sums the 4 partials per batch.

Per-core design (v2, chunk-pipelined):
  - Q^T/K^T [a,t] f32r via projection; V natural [s,a] bf16 with a ones
    column at a=64 so the AV matmul also emits softmax row-sums l[t].
  - Scores S^T[s,t] f32r, causally trimmed (diagonal s-blocks use moving
    widths 512/384/256/256 to stay >=256 for the f32r fast path).
  - exp on the Activation engine (psum -> sbuf bf16); triangle masking only
    on the 4 diagonal 128x128 blocks per (head, chunk) via one bf16 mask.
  - AV in NATURAL layout: out[t,65] += P^T-block^T @ V-block, ap=65 bf16
    (cost-model-optimal: rows = y entries / 128).  l[t] lands per-partition,
    so normalization is a DVE reciprocal + multiply (no ones-broadcast
    matmul).
  - y[t, c_local] is PE-transposed (128x128 blocks) into y^T for the output
    projection out[t,:] = y^T.T @ w_o.
  - Work is pipelined over 4 t-chunks of 512: QKV projection of chunk n+1
    and output projection of chunk n-1 are interleaved into chunk n's
    attention emission so the exp stream hides under PE work.
"""

import math
import os
import sys

import numpy as np

for _p in ("/opt/trn_rl_repo",):
    if _p not in sys.path:
        sys.path.insert(0, _p)

import ml_dtypes
import concourse.bass as bass
from concourse import bacc
import concourse.mybir as mybir
from concourse.bass import ts
from concourse.tile import TileContext
from concourse.bass_utils import run_bass_kernel_spmd
from contextlib import ExitStack

F32 = mybir.dt.float32
F32R = mybir.dt.float32r
BF16 = mybir.dt.bfloat16
AF = mybir.ActivationFunctionType

B, T, C = 2, 2048, 1024
H, CA = 16, 64
SCALE = 1.0 / math.sqrt(CA)
NCORES = 8
HPC = 4          # heads per core
TB = T // 128    # 16 t-blocks of 128
TC = T // 512    # 4 t-chunks of 512
CK = C // 128    # 8 c-chunks


def build_nc():
    nc = bacc.Bacc()
    xT = nc.declare_dram_parameter("xT", [CK, 128, T], F32R, isOutput=False)
    wq_d = nc.declare_dram_parameter("wq", [2, CK, 128, 128], F32R, isOutput=False)
    wk_d = nc.declare_dram_parameter("wk", [2, CK, 128, 128], F32R, isOutput=False)
    wv_d = nc.declare_dram_parameter("wv", [CK, 128, 2 * 128], F32R, isOutput=False)
    wo_d = nc.declare_dram_parameter("wo", [2, 128, C], F32R, isOutput=False)
    mask_d = nc.declare_dram_parameter("mask", [128, 128], BF16, isOutput=False)
    ident_d = nc.declare_dram_parameter("ident", [128, 128], F32R, isOutput=False)
    out = nc.declare_dram_parameter("out", [TB, 128, C], F32, isOutput=True)

    with TileContext(nc) as tc, ExitStack() as ctx:
        const = ctx.enter_context(tc.tile_pool(name="const", bufs=1))
        persist = ctx.enter_context(tc.tile_pool(name="persist", bufs=1))
        qpool = ctx.enter_context(tc.tile_pool(name="qpool", bufs=2))
        xpool = ctx.enter_context(tc.tile_pool(name="xpool", bufs=2))
        pbp = ctx.enter_context(tc.tile_pool(name="pbp", bufs=4))
        rcp = ctx.enter_context(tc.tile_pool(name="rcp", bufs=2))
        otp = ctx.enter_context(tc.tile_pool(name="otp", bufs=2))
        ps_s = ctx.enter_context(tc.tile_pool(name="ps_s", bufs=2, space="PSUM"))
        ps_y = ctx.enter_context(tc.tile_pool(name="ps_y", bufs=2, space="PSUM"))
        po = ctx.enter_context(tc.tile_pool(name="po", bufs=2, space="PSUM"))

        # ---- constants / weights: issue spread over idle engines, in the
        # order the first QKV units consume them (q p0, k p0, q p1, k p1, v)
        wq_sb = const.tile([128, 2, CK, 128], F32R, tag="wq")
        wk_sb = const.tile([128, 2, CK, 128], F32R, tag="wk")
        nc.sync.dma_start(wq_sb[:, 0], wq_d[0].rearrange("c r f -> r c f"))
        nc.scalar.dma_start(wk_sb[:, 0], wk_d[0].rearrange("c r f -> r c f"))
        nc.scalar.dma_start(wq_sb[:, 1], wq_d[1].rearrange("c r f -> r c f"))
        nc.scalar.dma_start(wk_sb[:, 1], wk_d[1].rearrange("c r f -> r c f"))
        wv_sb = const.tile([128, CK, 256], F32R, tag="wv")
        nc.gpsimd.dma_start(wv_sb[:], wv_d[:].rearrange("c r f -> r c f"))
        mask_sb = const.tile([128, 128], BF16, tag="mask")
        nc.scalar.dma_start(mask_sb[:], mask_d[:])
        ident_sb = const.tile([128, 128], F32R, tag="ident")
        nc.gpsimd.dma_start(ident_sb[:], ident_d[:])
        wo_sb = const.tile([128, 2, C], F32R, tag="wo")
        nc.gpsimd.dma_start(wo_sb[:], wo_d[:].rearrange("c r f -> r c f"))

        # ---- persistent state ----
        k_sbT = persist.tile([128, 2, T], F32R, tag="kT")
        v_sb = persist.tile([128, HPC, TB, 65], BF16, tag="v")
        nc.vector.memset(v_sb[:, :, :, 64:65], 1.0)
        y_sbT = persist.tile([128, 2, T], F32R, tag="yT")
        y_norm = persist.tile([128, TC, 4, 256], F32R, tag="ynorm")

        # ---- x chunks: chunk 0 split per-ck for fast start ----
        xs_tiles = [None] * TC
        xs_tiles[0] = xpool.tile([128, CK, 512], F32R, tag="xs", name="xs0")
        for ck in range(CK):
            nc.sync.dma_start(xs_tiles[0][:, ck, :], xT[ck, :, 0:512])

        def load_xs(tcn):
            xs_tiles[tcn] = xpool.tile([128, CK, 512], F32R, tag="xs",
                                       name=f"xs{tcn}")
            nc.gpsimd.dma_start(
                xs_tiles[tcn][:],
                xT[:, :, ts(tcn, 512)].rearrange("c r t -> r c t"),
            )

        q_tiles = [None] * TC

        def qkv_units(tcn):
            """PE filler units producing Q^T/K^T/V for chunk tcn."""
            xs = xs_tiles[tcn]
            q_tiles[tcn] = qpool.tile([128, 2, 512], F32R, tag="q",
                                      name=f"q{tcn}")
            units = []

            def qk_unit(p, w_sb, is_q):
                def u():
                    t = po.tile([128, 512], F32, tag="po", name="po_qk")
                    for ck in range(CK):
                        nc.tensor.matmul(
                            t[:], lhsT=w_sb[:, p, ck, :], rhs=xs[:, ck, :],
                            start=(ck == 0), stop=(ck == CK - 1),
                        )
                    if is_q:
                        nc.vector.tensor_copy(q_tiles[tcn][:, p, :], t[:])
                    else:
                        nc.vector.tensor_copy(k_sbT[:, p, ts(tcn, 512)], t[:])
                return u

            def v_unit(tbl):
                def u():
                    t = po.tile([128, 256], F32, tag="po", name="po_v")
                    for ck in range(CK):
                        nc.tensor.matmul(
                            t[:], lhsT=xs[:, ck, ts(tbl, 128)],
                            rhs=wv_sb[:, ck, :],
                            start=(ck == 0), stop=(ck == CK - 1),
                        )
                    nc.vector.tensor_copy(
                        v_sb[:, :, 4 * tcn + tbl, 0:64],
                        t[:].rearrange("r (h a) -> r h a", h=HPC),
                    )
                return u

            for p in range(2):
                units.append(qk_unit(p, wq_sb, True))
                units.append(qk_unit(p, wk_sb, False))
            for tbl in range(4):
                units.append(v_unit(tbl))
            return units

        def out_units(tcn):
            """PE filler units: transpose y(tcn) and project through w_o."""
            units = []

            def tr_unit(tbl):
                def u():
                    tb = 4 * tcn + tbl
                    t = po.tile([128, 2, 128], F32R, tag="po", name="po_tr")
                    for cj in range(2):
                        nc.tensor.transpose(
                            t[:, cj, :],
                            y_norm[:, tcn, tbl, ts(cj, 128)],
                            ident_sb[:],
                        )
                    nc.vector.tensor_copy(y_sbT[:, :, ts(tb, 128)], t[:])
                return u

            def o_unit(tbl):
                def u():
                    tb = 4 * tcn + tbl
                    ot = otp.tile([128, C], F32, tag="ot", name="ot")
                    for cc in range(2):
                        t = po.tile([128, 512], F32, tag="po", name="po_o")
                        for cj in range(2):
                            nc.tensor.matmul(
                                t[:], lhsT=y_sbT[:, cj, ts(tb, 128)],
                                rhs=wo_sb[:, cj, ts(cc, 512)],
                                start=(cj == 0), stop=(cj == 1),
                            )
                        nc.vector.tensor_copy(ot[:, ts(cc, 512)], t[:])
                    nc.sync.dma_start(out[tb], ot[:])
                return u

            for tbl in range(4):
                units.append(tr_unit(tbl))
                units.append(o_unit(tbl))
            return units

        # ---------------- the chunk pipeline ----------------
        start_units = qkv_units(0)
        for u in start_units:
            u()
        load_xs(1)

        for tcn in range(TC):
            fillers = []
            if tcn + 1 < TC:
                fillers += qkv_units(tcn + 1)
            if tcn == 2:
                fillers += out_units(0)
            if tcn == 3:
                fillers += out_units(1) + out_units(2)
            if tcn + 2 < TC:
                load_xs(tcn + 2)

            npairs = 2 * tcn + 2
            nslots = 4 * npairs
            stride = max(1, nslots // max(1, len(fillers)))
            slot = 0

            for h in range(HPC):
                p, hl = h // 2, h % 2
                b0 = 64 * hl
                q_ap = q_tiles[tcn]
                yt = ps_y.tile([128, 4, 65], F32, tag="y", name="yps")
                pb_tiles = []

                def av_emit(pi):
                    """AV matmuls for pair pi (after its exp+mask)."""
                    pb = pb_tiles[pi]
                    if pi < 2 * tcn:
                        sbs = [(2 * pi, lambda tbl: 128 * tbl, 0),
                               (2 * pi + 1, lambda tbl: 512 + 128 * tbl, 0)]
                    elif pi == 2 * tcn:
                        sbs = [(4 * tcn, lambda tbl: 128 * tbl, 0),
                               (4 * tcn + 1, lambda tbl: 384 + 128 * tbl, 1)]
                    else:
                        sbs = [(4 * tcn + 2, lambda tbl: 128 * tbl - 256, 2),
                               (4 * tcn + 3, lambda tbl: 128 * tbl, 3)]
                    # PSUM start/stop are zero-REGION (2KB bank) granular:
                    # arm lazy-zero exactly once per (head, chunk) on the
                    # first AV matmul; stop on the last emitted one.  Each
                    # tbl slice is zeroed on its first write after arming.
                    for sb, colf, tbl0 in sbs:
                        for tbl in range(tbl0, 4):
                            c0 = colf(tbl)
                            nc.tensor.matmul(
                                yt[:, tbl, :],
                                lhsT=pb[:, c0:c0 + 128],
                                rhs=v_sb[:, h, sb, :],
                                start=(sb == 0 and tbl == 0),
                                stop=(sb == 4 * tcn + 3 and tbl == 3),
                                skip_group_check=True,
                            )

                for pi in range(npairs):
                    sps = ps_s.tile([128, 1024], F32, tag="s", name="sps")
                    pb = pbp.tile([128, 1024], BF16, tag="pb", name="pb")
                    pb_tiles.append(pb)
                    if pi < 2 * tcn:
                        # two full 512-wide score blocks
                        for j in range(2):
                            sb = 2 * pi + j
                            nc.tensor.matmul(
                                sps[:, ts(j, 512)],
                                lhsT=k_sbT[b0:b0 + 64, p, ts(sb, 128)],
                                rhs=q_ap[b0:b0 + 64, p, :],
                                start=True, stop=True,
                            )
                        nc.scalar.activation(pb[:], sps[:], AF.Exp, scale=SCALE)
                    elif pi == 2 * tcn:
                        # diagonal pair A: d0 (512 wide) + d1 (384 wide)
                        nc.tensor.matmul(
                            sps[:, 0:512],
                            lhsT=k_sbT[b0:b0 + 64, p, ts(4 * tcn, 128)],
                            rhs=q_ap[b0:b0 + 64, p, :],
                            start=True, stop=True,
                        )
                        nc.tensor.matmul(
                            sps[:, 512:896],
                            lhsT=k_sbT[b0:b0 + 64, p, ts(4 * tcn + 1, 128)],
                            rhs=q_ap[b0:b0 + 64, p, 128:512],
                            start=True, stop=True,
                        )
                        nc.scalar.activation(pb[:, 0:896], sps[:, 0:896],
                                             AF.Exp, scale=SCALE)
                        nc.gpsimd.tensor_mul(pb[:, 0:128], pb[:, 0:128],
                                             mask_sb[:])
                        nc.gpsimd.tensor_mul(pb[:, 512:640], pb[:, 512:640],
                                             mask_sb[:])
                    else:
                        # diagonal pair B: d2 + d3 (256 wide each)
                        for j in range(2):
                            nc.tensor.matmul(
                                sps[:, ts(j, 256)],
                                lhsT=k_sbT[b0:b0 + 64, p,
                                           ts(4 * tcn + 2 + j, 128)],
                                rhs=q_ap[b0:b0 + 64, p, 256:512],
                                start=True, stop=True,
                            )
                        nc.scalar.activation(pb[:, 0:512], sps[:, 0:512],
                                             AF.Exp, scale=SCALE)
                        nc.gpsimd.tensor_mul(pb[:, 0:128], pb[:, 0:128],
                                             mask_sb[:])
                        nc.gpsimd.tensor_mul(pb[:, 384:512], pb[:, 384:512],
                                             mask_sb[:])
                    if pi >= 1:
                        av_emit(pi - 1)
                    slot += 1
                    if fillers and slot % stride == 0:
                        fillers.pop(0)()
                av_emit(npairs - 1)

                # normalize: y[t, a] /= l[t]  (l = col 64, per-partition)
                rc = rcp.tile([128, 4, 64], F32, tag="rc", name="rc")
                nc.vector.reciprocal(
                    rc[:], yt[:, :, 64:65].broadcast_to([128, 4, 64]))
                nc.vector.tensor_mul(
                    y_norm[:, tcn, :, 64 * h:64 * h + 64],
                    yt[:, :, 0:64], rc[:],
                )
            # drain any remaining fillers for this chunk
            for u in fillers:
                u()

        for u in out_units(3):
            u()

    nc.compile()
    return nc


_NC = None


def _get_nc():
    global _NC
    if _NC is None:
        _NC = build_nc()
    return _NC


def _mask_arr():
    p = np.arange(128)[:, None]
    f = np.arange(128)[None, :]
    return np.ascontiguousarray((p <= f).astype(ml_dtypes.bfloat16))


def make_in_maps(x, w_q, w_k, w_v, w_o):
    x = np.asarray(x, dtype=np.float32)
    w_q = np.asarray(w_q, dtype=np.float32)
    w_k = np.asarray(w_k, dtype=np.float32)
    w_v = np.asarray(w_v, dtype=np.float32)
    w_o = np.asarray(w_o, dtype=np.float32)
    in_maps = []
    for c in range(NCORES):
        b, g = c // 4, c % 4
        hs = [4 * g + i for i in range(HPC)]
        xT = np.ascontiguousarray(x[b].T).reshape(CK, 128, T)
        wq_a = np.stack([
            np.concatenate([w_q[hs[2 * p]], w_q[hs[2 * p + 1]]], axis=1).reshape(CK, 128, 128)
            for p in range(2)
        ])
        wk_a = np.stack([
            np.concatenate([w_k[hs[2 * p]], w_k[hs[2 * p + 1]]], axis=1).reshape(CK, 128, 128)
            for p in range(2)
        ])
        wv_a = np.concatenate([w_v[h] for h in hs], axis=1).reshape(CK, 128, 256)
        wo_a = w_o[256 * g:256 * (g + 1)].reshape(2, 128, C)
        in_maps.append(dict(
            mask=_mask_arr(),
            ident=np.ascontiguousarray(np.eye(128, dtype=np.float32)),
            xT=np.ascontiguousarray(xT),
            wq=np.ascontiguousarray(wq_a),
            wk=np.ascontiguousarray(wk_a),
            wv=np.ascontiguousarray(wv_a),
            wo=np.ascontiguousarray(wo_a),
        ))
    return in_maps


def gather_out(results):
    acc = [np.zeros((T, C), np.float64) for _ in range(B)]
    for c in range(NCORES):
        acc[c // 4] += results[c]["out"].reshape(T, C).astype(np.float64)
    return np.stack([a.astype(np.float32) for a in acc])


def run(x, w_q, w_k, w_v, w_o, trace=False, **spmd_kwargs):
    nc = _get_nc()
    in_maps = make_in_maps(x, w_q, w_k, w_v, w_o)
    res = run_bass_kernel_spmd(nc, in_maps, list(range(NCORES)), trace=trace,
                               **spmd_kwargs)
    return gather_out(res.results), res


def kernel(x, w_q, w_k, w_v, w_o):
    out, _ = run(x, w_q, w_k, w_v, w_o)
    return out
